# revision 3
# baseline (speedup 1.0000x reference)
"""AttnBlock1D (BN + single-head 1x1-conv attention + residual) on 8 TRN2 cores.

Contract: kernel(**inputs) takes the FULL inputs from setup_inputs() and
returns the FULL output [4, 256, 4096] f32.

Sharding: 8 cores = 4 samples x 2 query-halves. Core i handles sample
b = i // 2 and queries [qh*2048, (qh+1)*2048) with qh = i % 2. The host
rolls x[b] along L so each core's queries are the FIRST 2048 columns --
attention is permutation-invariant over keys, so k/v built from the rolled
layout give identical softmax results. This keeps the SPMD program free of
per-core constants.

BatchNorm stats are synced with one [256, 2] AllReduce of per-core
(mean, E[x^2]) (every sample counted twice -- uniform, so /8 is exact).

Matmul dtype is fp16 (1 cycle/row on the PE like bf16, but 11-bit
mantissa). PSUM accumulation is fp32. Attention scores are computed
transposed (ST[j, i] = sum_c k[c, j] q[c, i]) so the probabilities land
with j (keys) on the partition axis, which the AV matmul contracts
natively; softmax has no max-subtraction (scores ~ N(0, 1), exp is safe)
and the denominator comes from an extra ones[128,128] matmul that also
broadcasts it across partitions. The v-bias is folded into the output
projection bias on the host (wp @ bv) since softmax rows sum to one.
"""

import os

import numpy as np

import concourse.bass as bass
import concourse.mybir as mybir
import concourse.tile as tile
from concourse import bacc
from concourse import bass_utils

F32 = mybir.dt.float32
F16 = mybir.dt.float16

N_CORES = 8
B, C, L = 4, 256, 4096
M = L // 2          # queries per core
EPS = 1e-5
SCALE = 1.0 / 16.0  # C ** -0.5

NCHUNK = 4          # query chunks per core
CH = M // NCHUNK    # 512 queries per chunk
NJT = L // 128      # 32 key tiles

LAST_EXEC_NS = None
_COMPILED = None


def _build():
    nc = bacc.Bacc("TRN2", target_bir_lowering=False, debug=False,
                   num_devices=N_CORES)

    x_d = nc.dram_tensor("x", [C, L], F32, kind="ExternalInput")
    wq_d = nc.dram_tensor("wqT", [C, C], F16, kind="ExternalInput")
    wk_d = nc.dram_tensor("wkT", [C, C], F16, kind="ExternalInput")
    wv_d = nc.dram_tensor("wvT", [C, C], F16, kind="ExternalInput")
    wp_d = nc.dram_tensor("wpT", [C, C], F16, kind="ExternalInput")
    bq_d = nc.dram_tensor("bq", [C, 1], F32, kind="ExternalInput")
    bk_d = nc.dram_tensor("bk", [C, 1], F32, kind="ExternalInput")
    bp_d = nc.dram_tensor("bpe", [C, 1], F32, kind="ExternalInput")
    gam_d = nc.dram_tensor("gamma", [C, 1], F32, kind="ExternalInput")
    bet_d = nc.dram_tensor("beta", [C, 1], F32, kind="ExternalInput")
    out_d = nc.dram_tensor("out", [C, M], F32, kind="ExternalOutput")

    cc_in = nc.dram_tensor("cc_in", [C, 2], F32, kind="Internal")
    cc_out = nc.dram_tensor("cc_out", [C, 2], F32, kind="Internal",
                            addr_space="Shared")

    with tile.TileContext(nc) as tc:
        with (
            tc.tile_pool(name="big", bufs=1) as big,
            tc.tile_pool(name="pt", bufs=2) as ptp,
            tc.tile_pool(name="small", bufs=2) as sm,
            tc.tile_pool(name="eps", bufs=3) as epi,
            tc.tile_pool(name="ps_s", bufs=2, space="PSUM") as ps_s,
            tc.tile_pool(name="ps_acc", bufs=1, space="PSUM") as ps_acc,
            tc.tile_pool(name="ps_o", bufs=1, space="PSUM") as ps_o,
        ):
            # ---------------- load x, weights, vectors ----------------
            x_t = [big.tile([128, L], F32, name=f"x{h}") for h in range(2)]
            for h in range(2):
                nc.sync.dma_start(x_t[h][:], x_d[h * 128:(h + 1) * 128, :])

            w_t = {}
            for nm, d in (("q", wq_d), ("k", wk_d), ("v", wv_d), ("p", wp_d)):
                w_t[nm] = [big.tile([128, C], F16, name=f"w{nm}{h}")
                           for h in range(2)]
                for h in range(2):
                    nc.sync.dma_start(w_t[nm][h][:], d[h * 128:(h + 1) * 128, :])

            vecs = {}
            for nm, d in (("bq", bq_d), ("bk", bk_d), ("bpe", bp_d),
                          ("gam", gam_d), ("bet", bet_d)):
                vecs[nm] = [big.tile([128, 1], F32, name=f"{nm}{h}")
                            for h in range(2)]
                for h in range(2):
                    nc.sync.dma_start(vecs[nm][h][:], d[h * 128:(h + 1) * 128, :])

            # ---------------- BN stats + sync ----------------
            st_t = []
            for h in range(2):
                s6 = sm.tile([128, 48], F32, name=f"s6_{h}")
                for i in range(8):
                    nc.vector.bn_stats(
                        s6[:, i * 6:(i + 1) * 6],
                        x_t[h][:, i * 512:(i + 1) * 512],
                    )
                s2 = sm.tile([128, 2], F32, name=f"s2_{h}")
                nc.vector.bn_aggr(s2[:], s6[:])
                # payload: (mean, E[x^2] = var + mean^2)
                pay = sm.tile([128, 2], F32, name=f"pay{h}")
                nc.vector.tensor_copy(pay[:, 0:1], s2[:, 0:1])
                m2 = sm.tile([128, 1], F32, name=f"m2_{h}")
                nc.vector.tensor_mul(m2[:], s2[:, 0:1], s2[:, 0:1])
                nc.vector.tensor_add(pay[:, 1:2], s2[:, 1:2], m2[:])
                nc.sync.dma_start(cc_in[h * 128:(h + 1) * 128, :], pay[:])
                st_t.append(pay)

            nc.gpsimd.collective_compute(
                "AllReduce",
                mybir.AluOpType.add,
                replica_groups=[list(range(N_CORES))],
                ins=[cc_in[:]],
                outs=[cc_out[:]],
            )

            a_t, d_t = [], []
            for h in range(2):
                g = sm.tile([128, 2], F32, name=f"g{h}")
                nc.sync.dma_start(g[:], cc_out[h * 128:(h + 1) * 128, :])
                gm = sm.tile([128, 1], F32, name=f"gm{h}")
                nc.vector.tensor_scalar_mul(gm[:], g[:, 0:1], 1.0 / N_CORES)
                ge2 = sm.tile([128, 1], F32, name=f"ge2{h}")
                nc.vector.tensor_scalar_mul(ge2[:], g[:, 1:2], 1.0 / N_CORES)
                mm = sm.tile([128, 1], F32, name=f"mm{h}")
                nc.vector.tensor_mul(mm[:], gm[:], gm[:])
                var = sm.tile([128, 1], F32, name=f"var{h}")
                nc.vector.tensor_sub(var[:], ge2[:], mm[:])
                nc.vector.tensor_scalar_add(var[:], var[:], EPS)
                sd = sm.tile([128, 1], F32, name=f"sd{h}")
                nc.scalar.activation(sd[:], var[:],
                                     mybir.ActivationFunctionType.Sqrt)
                rs = sm.tile([128, 1], F32, name=f"rs{h}")
                nc.vector.reciprocal(rs[:], sd[:])
                a = sm.tile([128, 1], F32, name=f"a{h}")
                nc.vector.tensor_mul(a[:], rs[:], vecs["gam"][h][:])
                ma = sm.tile([128, 1], F32, name=f"ma{h}")
                nc.vector.tensor_mul(ma[:], gm[:], a[:])
                dd = sm.tile([128, 1], F32, name=f"d{h}")
                nc.vector.tensor_sub(dd[:], vecs["bet"][h][:], ma[:])
                a_t.append(a)
                d_t.append(dd)

            # ---------------- normalize: h = x*a + d (fp16) ----------------
            h_t = [big.tile([128, L], F16, name=f"h{h}") for h in range(2)]
            for h in range(2):
                nc.vector.tensor_scalar(
                    out=h_t[h][:], in0=x_t[h][:],
                    scalar1=a_t[h][:], scalar2=d_t[h][:],
                    op0=mybir.AluOpType.mult, op1=mybir.AluOpType.add,
                )

            # ---------------- projections ----------------
            q_t = [big.tile([128, M], F16, name=f"q{h}") for h in range(2)]
            k_t = [big.tile([128, L], F16, name=f"k{h}") for h in range(2)]
            vT_t = big.tile([128, NJT * 256], F16, name="vT")

            # q: only first M columns of (rolled) h
            for oh in range(2):
                for it in range(M // 512):
                    ps = ps_s.tile([128, 512], F32, tag="s", name="ps_q")
                    for ch in range(2):
                        nc.tensor.matmul(
                            ps[:],
                            w_t["q"][ch][:, oh * 128:(oh + 1) * 128],
                            h_t[ch][:, it * 512:(it + 1) * 512],
                            start=(ch == 0), stop=(ch == 1),
                        )
                    nc.vector.tensor_scalar_add(
                        q_t[oh][:, it * 512:(it + 1) * 512], ps[:],
                        vecs["bq"][oh][:])

            # k: all L columns
            for oh in range(2):
                for it in range(L // 512):
                    ps = ps_s.tile([128, 512], F32, tag="s", name="ps_k")
                    for ch in range(2):
                        nc.tensor.matmul(
                            ps[:],
                            w_t["k"][ch][:, oh * 128:(oh + 1) * 128],
                            h_t[ch][:, it * 512:(it + 1) * 512],
                            start=(ch == 0), stop=(ch == 1),
                        )
                    nc.vector.tensor_scalar_add(
                        k_t[oh][:, it * 512:(it + 1) * 512], ps[:],
                        vecs["bk"][oh][:])

            # vT: [l, o] tiles (bias folded into bpe on host)
            for lt in range(NJT):
                ps = ps_s.tile([128, 512], F32, tag="s", name="ps_v")
                for ch in range(2):
                    nc.tensor.matmul(
                        ps[:, 0:256],
                        h_t[ch][:, lt * 128:(lt + 1) * 128],
                        w_t["v"][ch][:],
                        start=(ch == 0), stop=(ch == 1),
                    )
                nc.vector.tensor_copy(
                    vT_t[:, lt * 256:(lt + 1) * 256], ps[:, 0:256])

            ones_t = big.tile([128, 128], F16, name="ones")
            nc.vector.memset(ones_t[:], 1.0)

            # ---------------- attention, chunk by chunk ----------------
            for cn in range(NCHUNK):
                i0 = cn * CH
                # scores (transposed) + exp -> pT
                pT = ptp.tile([128, NJT * CH], F16, tag="pT", name=f"pT{cn}")
                for jp in range(NJT // 2):
                    ps = ps_s.tile([128, 1024], F32, tag="s", name="ps_sc")
                    for half in range(2):
                        jt = jp * 2 + half
                        for ch in range(2):
                            nc.tensor.matmul(
                                ps[:, half * 512:(half + 1) * 512],
                                k_t[ch][:, jt * 128:(jt + 1) * 128],
                                q_t[ch][:, i0:i0 + CH],
                                start=(ch == 0), stop=(ch == 1),
                            )
                    nc.scalar.activation(
                        pT[:, jp * 1024:(jp + 1) * 1024], ps[:],
                        mybir.ActivationFunctionType.Exp, scale=SCALE)

                # AV + denominator accumulation over all key tiles
                ps_av = [ps_acc.tile([128, CH], F32, tag=f"av{ch}",
                                     name=f"av{ch}_{cn}") for ch in range(2)]
                ps_den = ps_acc.tile([128, CH], F32, tag="den",
                                     name=f"den{cn}")
                for jt in range(NJT):
                    pslice = pT[:, jt * CH:(jt + 1) * CH]
                    for ch in range(2):
                        nc.tensor.matmul(
                            ps_av[ch][:],
                            vT_t[:, jt * 256 + ch * 128:jt * 256 + (ch + 1) * 128],
                            pslice,
                            start=(jt == 0), stop=(jt == NJT - 1),
                        )
                    nc.tensor.matmul(
                        ps_den[:], ones_t[:], pslice,
                        start=(jt == 0), stop=(jt == NJT - 1),
                    )

                rec = epi.tile([128, CH], F16, tag="rec", name=f"rec{cn}")
                with nc.allow_low_precision(
                        reason="softmax denom recip: f16 rel err 2^-11 ok"):
                    nc.vector.reciprocal(rec[:], ps_den[:])

                at_t = []
                for ch in range(2):
                    at = epi.tile([128, CH], F16, tag=f"at{ch}",
                                  name=f"at{ch}_{cn}")
                    nc.vector.tensor_mul(at[:], ps_av[ch][:], rec[:])
                    at_t.append(at)

                # output projection + bias + residual
                for oh in range(2):
                    ps = ps_o.tile([128, CH], F32, tag="o", name=f"po{oh}_{cn}")
                    for ch in range(2):
                        nc.tensor.matmul(
                            ps[:],
                            w_t["p"][ch][:, oh * 128:(oh + 1) * 128],
                            at_t[ch][:],
                            start=(ch == 0), stop=(ch == 1),
                        )
                    res = epi.tile([128, CH], F32, tag="res", name=f"res{oh}_{cn}")
                    nc.vector.scalar_tensor_tensor(
                        out=res[:], in0=ps[:], scalar=vecs["bpe"][oh][:],
                        in1=x_t[oh][:, i0:i0 + CH],
                        op0=mybir.AluOpType.add, op1=mybir.AluOpType.add,
                    )
                    nc.sync.dma_start(
                        out_d[oh * 128:(oh + 1) * 128, i0:i0 + CH], res[:])

    nc.compile()
    return nc


def kernel(x, gamma, beta, wq, bq, wk, bk, wv, bv, wp, bp):
    global _COMPILED, LAST_EXEC_NS
    x = np.asarray(x, np.float32)
    if _COMPILED is None:
        _COMPILED = _build()
    nc = _COMPILED

    common = {
        "wqT": np.ascontiguousarray(np.asarray(wq, np.float32).T).astype(np.float16),
        "wkT": np.ascontiguousarray(np.asarray(wk, np.float32).T).astype(np.float16),
        "wvT": np.ascontiguousarray(np.asarray(wv, np.float32).T).astype(np.float16),
        "wpT": np.ascontiguousarray(np.asarray(wp, np.float32).T).astype(np.float16),
        "bq": np.asarray(bq, np.float32).reshape(C, 1),
        "bk": np.asarray(bk, np.float32).reshape(C, 1),
        "bpe": (np.asarray(bp, np.float32)
                + np.asarray(wp, np.float32) @ np.asarray(bv, np.float32)
                ).reshape(C, 1),
        "gamma": np.asarray(gamma, np.float32).reshape(C, 1),
        "beta": np.asarray(beta, np.float32).reshape(C, 1),
    }

    in_maps = []
    for core in range(N_CORES):
        b, qh = core // 2, core % 2
        xb = x[b]
        if qh:
            xb = np.ascontiguousarray(np.roll(xb, -M, axis=1))
        in_maps.append({"x": xb, **common})

    trace = os.environ.get("BASS_KERNEL_TRACE", "") == "1"
    res = bass_utils.run_bass_kernel_spmd(
        nc, in_maps, core_ids=list(range(N_CORES)), trace=trace)
    LAST_EXEC_NS = res.exec_time_ns

    out = np.empty((B, C, L), np.float32)
    for core in range(N_CORES):
        b, qh = core // 2, core % 2
        out[b, :, qh * M:(qh + 1) * M] = res.results[core]["out"]
    return out
